# revision 30
# baseline (speedup 1.0000x reference)
"""MoE top-1 feed-forward (DeepSpeed-style) on 8 Trainium2 NeuronCores.

Strategy (expert parallelism, per the sharding hint):
  - Host computes the (tiny) gate: logits = x @ Wg, softmax, top-1 expert id
    and gate prob per token (float64 for a faithful argmax).
  - Tokens are dispatched to the core owning their expert (core e holds
    W1[e]/b1[e]/W2[e]/b2[e]); each core's token batch is padded to a common
    capacity C so all 8 cores run one SPMD program.
  - Each core runs the dense FFN for its tokens in bf16 (f32 PSUM):
        phase 1:  hT[f] = silu(sum_d W1[d,f]^T @ xT[d] + b1)   f = 0..KF-1
        phase 2:  yT[d] = sum_f W2[f,d]^T @ hT[f]              d = 0..KD-1
    Tokens ride the free (moving) dimension, so no on-device transposes.
    The dd-major phase 2 lets each yT[d] drain (PSUM->SBUF->HBM) while the
    next accumulates, shrinking the post-matmul tail to one chunk.
  - All DRAM images are host-packed partition-major and contiguous so every
    DMA has large (>=3KB) per-partition descriptors: small-descriptor
    transfers round-robin badly against the bulk weight traffic (measured
    15us of lead-in on a rearranged x load).
  - Ring split: sync carries x then the W1 group stream; scalar carries b1
    then W2 pair-tiles paced into the mm1 phase so the W1 stream (the
    critical one) is never starved; y chunks stream out on sync.
  - A short burst of dummy matmuls on a memset scratch tile warms the PE
    clock (HAM 1.2->2.4 GHz takes ~3.4us of activity) while x/W1 land.
  - Host combines: out[token] = gate * (y + b2[expert]).
"""

import os
import sys

import numpy as np

try:
    import concourse.mybir as mybir  # noqa: F401
except ModuleNotFoundError:  # fallback if the site hooks aren't installed
    sys.path.insert(0, "/opt/trn_rl_repo")

import concourse.mybir as mybir
import concourse.tile as tile
from concourse import bacc
from concourse.bass_utils import run_bass_kernel_spmd

N_CORES = 8

# Compute dtype for the matmuls:
#   "bf16" - weights/activations cast to bfloat16 (f32 PSUM accumulate)
#   "f32r" - fp32 data, PE's replicated-fp32 mode (full rate at N>=256)
#   "f32"  - plain fp32 matmuls (4x slower PE)
MODE = os.environ.get("BASS_MOE_MODE", "bf16")

FG = int(os.environ.get("BASS_MOE_FG", "4"))  # steady-state f-chunks per W1 group
W2P = int(os.environ.get("BASS_MOE_W2P", "4"))  # f-chunks per W2 pair-tile
WARM = int(os.environ.get("BASS_MOE_WARM", "9"))  # dummy warmup matmuls (>=3.4us
# of continuous PE activity so the HAM clock-gate opens before the first stall)
CMAX = int(os.environ.get("BASS_MOE_CMAX", "256"))  # device capacity cap
LEADS = [int(v) for v in os.environ.get("BASS_MOE_LEADS", "1,1,2").split(",")]


def _w1_groups(KF):
    """F-chunk widths per W1 group: small leading groups let the PE start
    before the whole first FG-wide image lands, and keep per-chunk data
    availability fine-grained while the 8 cores' synchronized opening DMA
    burst halves the per-core HBM share (a group is usable only when its
    whole transfer lands)."""
    lead = list(LEADS) if FG > 2 and KF > 12 else []
    rem = KF - sum(lead)
    groups = list(lead)
    while rem > 0:
        w = min(FG, rem)
        groups.append(w)
        rem -= w
    return groups


_CACHE: dict = {}
_WCACHE: dict = {}


def _roundup(a: int, m: int) -> int:
    return -(-a // m) * m


def _build_bass(C: int, mode: str, D: int, F: int):
    """Build + compile the per-core Bass program for capacity C (<=512)."""
    f32 = mybir.dt.float32
    if mode == "bf16":
        dt_io = mybir.dt.bfloat16
    elif mode == "f32r":
        dt_io = mybir.dt.float32r
    else:
        dt_io = f32

    KD, KF = D // 128, F // 128
    GRPS = _w1_groups(KF)
    NP = -(-KF // W2P)  # number of W2 pair-tiles
    assert 256 <= C <= 512

    nc = bacc.Bacc(None, target_bir_lowering=False, debug=False)
    # Host-packed images (see kernel() for the packing). All contiguous
    # partition-major so per-partition descriptors are large.
    #   xP   [128, KD*C]    xP[p, d*C+c] = x[c, d*128+p]
    #   w1   [128, KD*F]    group images; group g at column offset
    #                       KD*128*sum(GRPS[:g]), blocks (d, j) within a
    #                       group at (d*gw+j)*128
    #   w2   [NP, 128, W2P*D]  pair p, f-chunk r=f-p*W2P at cols r*D
    #   b1r  [128, KF]      b1[f*128+p] at [p, f]
    #   yP   [128, KD*C]    output, same layout as xP (io dtype)
    xP = nc.dram_tensor("xP", [128, KD * C], dt_io, kind="ExternalInput")
    w1 = nc.dram_tensor("w1", [128, KD * F], dt_io, kind="ExternalInput")
    w2 = nc.dram_tensor("w2", [NP, 128, W2P * D], dt_io, kind="ExternalInput")
    b1r = nc.dram_tensor("b1r", [128, KF], f32, kind="ExternalInput")
    yP = nc.dram_tensor("yP", [128, KD * C], dt_io, kind="ExternalOutput")

    silu = mybir.ActivationFunctionType.Silu

    with tile.TileContext(nc) as tc:
        with (
            tc.tile_pool(name="xp", bufs=1) as xp,
            tc.tile_pool(name="w1p", bufs=4) as w1p,
            tc.tile_pool(name="w2p", bufs=1) as w2p,
            tc.tile_pool(name="hp", bufs=1) as hp,
            tc.tile_pool(name="bp", bufs=1) as bp,
            tc.tile_pool(name="yp", bufs=6) as yp,
            tc.tile_pool(name="wup", bufs=1) as wup,
            tc.tile_pool(name="ps_h", bufs=2, space="PSUM") as ps_h,
            tc.tile_pool(name="ps_y", bufs=1, space="PSUM") as ps_y,
        ):
            # PE warmup: dummy matmuls on a zeroed scratch tile keep the PE
            # busy (and ramping to 2.4 GHz) while the first x/W1 DMAs land.
            # The dummies borrow a ps_h buffer; the first real mm1 chunk's
            # start=True overwrites it (WAW-ordered on the in-order PE).
            if WARM > 0:
                wt = wup.tile([128, 512], dt_io, tag="warm", name="warm")
                nc.vector.memset(wt[:], 0.0)
                pw = ps_h.tile([128, 512], f32, tag="hps", name="pw")
                for _ in range(WARM):
                    nc.tensor.matmul(pw[:], wt[:, 0:128], wt[:], start=True, stop=True)

            # ALL bulk traffic rides the sync ring in consumption order
            # (xP, W1 g0..gN, W2 p0..pN): one ring drains FIFO at full
            # aggregate bandwidth, so nothing round-robins against the
            # critical W1 stream. Scalar carries only b1 (tiny) + y-outs.
            xt = xp.tile([128, KD * C], dt_io, tag="x", name="xt")
            nc.sync.dma_start(out=xt[:], in_=xP[:])
            b1t = bp.tile([128, KF], f32, tag="b1", name="b1t")
            nc.scalar.dma_start(out=b1t[:], in_=b1r[:])

            w2ts: list = [None] * NP

            def load_w2(p):
                t = w2p.tile([128, W2P * D], dt_io, tag=f"w2_{p}", name=f"w2t{p}")
                nc.sync.dma_start(out=t[:], in_=w2[p])
                w2ts[p] = t

            # phase 1: hT[f] = silu(W1[:, f]^T @ xT + b1) for all f
            hts: list = [None] * KF
            f0 = 0
            for g, gw in enumerate(GRPS):
                off = KD * 128 * f0
                w1g = w1p.tile(
                    [128, KD * gw * 128],
                    dt_io,
                    tag="w1g",
                    name=f"w1g{g}",
                    padded_shape=[128, KD * FG * 128],
                )
                nc.sync.dma_start(out=w1g[:], in_=w1[:, off : off + KD * gw * 128])
                for j in range(gw):
                    f = f0 + j
                    ph = ps_h.tile([128, C], f32, tag="hps", name="ph")
                    for d in range(KD):
                        nc.tensor.matmul(
                            ph[:],
                            w1g[:, (d * gw + j) * 128 : (d * gw + j + 1) * 128],
                            xt[:, d * C : (d + 1) * C],
                            start=(d == 0),
                            stop=(d == KD - 1),
                        )
                    ht = hp.tile([128, C], dt_io, tag=f"h{f}", name=f"ht{f}")
                    nc.scalar.activation(ht[:], ph[:], silu, bias=b1t[:, f : f + 1])
                    hts[f] = ht
                f0 += gw
            # W2 pairs emitted after the W1 stream: the scheduler issues the
            # sync ring's triggers in priority (= program) order, so these
            # queue behind every W1 group and drain in consumption order.
            for p in range(NP):
                load_w2(p)

            # phase 2: yT[d] = sum_f W2[f, d]^T @ hT[f].
            # Pair-major order so W2 pair p is first touched only after
            # p * KD * W2P * C PE-cycles of phase 2 — consumption deadlines
            # match the sync ring's FIFO arrival order with maximum slack
            # (dd-major would need ALL of W2 resident at phase-2 start).
            # Each d-chunk drains (PSUM->SBUF->HBM) as its accumulation
            # completes in the last pair's pass.
            pys = [
                ps_y.tile([128, C], f32, tag=f"y{dd}", name=f"py{dd}")
                for dd in range(KD)
            ]

            def mm2(p, dd, r):
                f = p * W2P + r
                nc.tensor.matmul(
                    pys[dd][:],
                    w2ts[p][:, r * D + dd * 128 : r * D + (dd + 1) * 128],
                    hts[f][:],
                    start=(f == 0),
                    stop=(f == KF - 1),
                )

            tail_pairs = 2 if NP >= 3 else 1
            for p in range(NP - tail_pairs):
                for dd in range(KD):
                    for r in range(W2P):
                        mm2(p, dd, r)
            # last pairs run dd-major so each d-chunk's accumulation finishes
            # (stop=True) a block early and its drain staggers under the
            # remaining matmuls
            for dd in range(KD):
                for p in range(NP - tail_pairs, NP):
                    for r in range(W2P):
                        mm2(p, dd, r)
                yt = yp.tile([128, C], dt_io, tag="yt", name=f"yt{dd}")
                # last d-chunk's evac on ACT so it overlaps DVE finishing
                # the previous one; out-DMA triggers alternate rings
                if dd == KD - 1:
                    nc.scalar.copy(yt[:], pys[dd][:])
                else:
                    nc.vector.tensor_copy(yt[:], pys[dd][:])
                out_eng = nc.scalar if dd % 2 == 0 else nc.sync
                out_eng.dma_start(out=yP[:, dd * C : (dd + 1) * C], in_=yt[:])

    nc.compile()
    return nc


def _get_bass(C: int, mode: str, D: int, F: int):
    key = (C, mode, D, F, FG, W2P, WARM, tuple(LEADS))
    if key not in _CACHE:
        _CACHE[key] = _build_bass(C, mode, D, F)
    return _CACHE[key]


def _gate_host(x: np.ndarray, Wg: np.ndarray):
    """Top-1 gating in float64: returns (expert_idx [T], gate [T] f32)."""
    logits = x.astype(np.float64) @ Wg.astype(np.float64)
    m = logits.max(-1, keepdims=True)
    p = np.exp(logits - m)
    p /= p.sum(-1, keepdims=True)
    return p.argmax(-1), p.max(-1).astype(np.float32)


def _kernel_numpy(x, Wg, W1, b1, W2, b2):
    """Reference-equivalent fallback (host only)."""
    idx, gate = _gate_host(x, Wg)
    out = np.zeros_like(x)
    for e in range(W1.shape[0]):
        ids = np.nonzero(idx == e)[0]
        if ids.size == 0:
            continue
        h = x[ids] @ W1[e] + b1[e]
        h = h * (1.0 / (1.0 + np.exp(-h)))
        out[ids] = gate[ids, None] * (h @ W2[e] + b2[e])
    return out


def _pack_weights(W1, b1, W2, np_io, D, F):
    """Per-expert weight images (cached across calls on array identity)."""
    key = (id(W1), id(W2), id(b1), np_io)
    ent = _WCACHE.get(key)
    if ent is not None and ent[0] is W1 and ent[1] is W2 and ent[2] is b1:
        return ent[3]
    E = W1.shape[0]
    KD, KF = D // 128, F // 128
    NP = -(-KF // W2P)
    grps = _w1_groups(KF)
    packed = []
    for e in range(E):
        w1e = W1[e].reshape(KD, 128, KF, 128)
        parts = []
        f0 = 0
        for gw in grps:
            blk = w1e[:, :, f0 : f0 + gw]  # [KD, 128, gw, 128]
            parts.append(blk.transpose(1, 0, 2, 3).reshape(128, KD * gw * 128))
            f0 += gw
        w1r = np.concatenate(parts, axis=1)  # [128, KD*F]
        w2r = (
            W2[e]
            .reshape(NP, W2P, 128, D)
            .transpose(0, 2, 1, 3)
            .reshape(NP, 128, W2P * D)
        )
        packed.append(
            {
                "w1": np.ascontiguousarray(w1r).astype(np_io, copy=False),
                "w2": np.ascontiguousarray(w2r).astype(np_io, copy=False),
                "b1r": np.ascontiguousarray(b1[e].reshape(KF, 128).T),
            }
        )
    _WCACHE[key] = (W1, W2, b1, packed)
    return packed


def kernel(hidden_states, Wg, W1, b1, W2, b2):
    hidden_states = np.asarray(hidden_states)
    Wg = np.asarray(Wg, dtype=np.float32)
    W1 = np.asarray(W1, dtype=np.float32)
    b1 = np.asarray(b1, dtype=np.float32)
    W2 = np.asarray(W2, dtype=np.float32)
    b2 = np.asarray(b2, dtype=np.float32)

    orig_shape = hidden_states.shape
    D = orig_shape[-1]
    x = np.ascontiguousarray(hidden_states, dtype=np.float32).reshape(-1, D)
    E, _, F = W1.shape
    KD, KF = D // 128, F // 128

    if E != N_CORES or D % 128 != 0 or F % 128 != 0:
        return _kernel_numpy(x, Wg, W1, b1, W2, b2).reshape(orig_shape)

    idx, gate = _gate_host(x, Wg)
    order = np.argsort(idx, kind="stable")
    counts = np.bincount(idx, minlength=E)
    starts = np.concatenate([[0], np.cumsum(counts)])

    # Capacity: common padded token count per core (single slab, <=512 to fit
    # one PSUM bank per matmul output). Capacity is capped at CMAX (the
    # balanced load T/E) and the few overflow tokens of hot experts are
    # FFN'd on the host — classic capacity-factor MoE with a residual
    # correction instead of token dropping.
    C = max(256, _roundup(int(counts.max()), 16))
    C = min(C, max(256, CMAX))
    if C > 512:
        return _kernel_numpy(x, Wg, W1, b1, W2, b2).reshape(orig_shape)

    mode = MODE
    np_io = np.float32
    if mode == "bf16":
        import ml_dtypes

        np_io = ml_dtypes.bfloat16

    nc = _get_bass(C, mode, D, F)
    wpack = _pack_weights(W1, b1, W2, np_io, D, F)

    in_maps = []
    for e in range(E):
        ids = order[starts[e] : starts[e + 1]][:C]
        xe = np.zeros((C, D), dtype=np.float32)
        xe[: ids.size] = x[ids]
        # xP[p, d*C+c] = xe[c, d*128+p]
        xPr = xe.reshape(C, KD, 128).transpose(2, 1, 0).reshape(128, KD * C)
        m = dict(wpack[e])
        m["xP"] = np.ascontiguousarray(xPr).astype(np_io, copy=False)
        in_maps.append(m)

    res = run_bass_kernel_spmd(nc, in_maps, list(range(N_CORES)))

    out = np.zeros_like(x)
    for e in range(E):
        ids = order[starts[e] : starts[e + 1]][:C]
        if ids.size:
            yr = np.asarray(res.results[e]["yP"], dtype=np.float32)  # [128, KD*C]
            y = yr.reshape(128, KD, C).transpose(2, 1, 0).reshape(C, D)[: ids.size]
            out[ids] = gate[ids, None] * (y + b2[e])
        over = order[starts[e] + C : starts[e + 1]]
        if over.size:  # host FFN for tokens past capacity
            h = x[over] @ W1[e] + b1[e]
            h = h * (1.0 / (1.0 + np.exp(-h)))
            out[over] = gate[over, None] * (h @ W2[e] + b2[e])
    return out.reshape(orig_shape)


# revision 31
# speedup vs baseline: 1.0175x; 1.0175x over previous
"""MoE top-1 feed-forward (DeepSpeed-style) on 8 Trainium2 NeuronCores.

Strategy (expert parallelism, per the sharding hint):
  - Host computes the (tiny) gate: logits = x @ Wg, softmax, top-1 expert id
    and gate prob per token (float64 for a faithful argmax).
  - Tokens are dispatched to the core owning their expert (core e holds
    W1[e]/b1[e]/W2[e]/b2[e]); each core's token batch is padded to a common
    capacity C so all 8 cores run one SPMD program.
  - Each core runs the dense FFN for its tokens in bf16 (f32 PSUM):
        phase 1:  hT[f] = silu(sum_d W1[d,f]^T @ xT[d] + b1)   f = 0..KF-1
        phase 2:  yT[d] = sum_f W2[f,d]^T @ hT[f]              d = 0..KD-1
    Tokens ride the free (moving) dimension, so no on-device transposes.
    The dd-major phase 2 lets each yT[d] drain (PSUM->SBUF->HBM) while the
    next accumulates, shrinking the post-matmul tail to one chunk.
  - All DRAM images are host-packed partition-major and contiguous so every
    DMA has large (>=3KB) per-partition descriptors: small-descriptor
    transfers round-robin badly against the bulk weight traffic (measured
    15us of lead-in on a rearranged x load).
  - Ring split: sync carries x then the W1 group stream; scalar carries b1
    then W2 pair-tiles paced into the mm1 phase so the W1 stream (the
    critical one) is never starved; y chunks stream out on sync.
  - A short burst of dummy matmuls on a memset scratch tile warms the PE
    clock (HAM 1.2->2.4 GHz takes ~3.4us of activity) while x/W1 land.
  - Host combines: out[token] = gate * (y + b2[expert]).
"""

import os
import sys

import numpy as np

try:
    import concourse.mybir as mybir  # noqa: F401
except ModuleNotFoundError:  # fallback if the site hooks aren't installed
    sys.path.insert(0, "/opt/trn_rl_repo")

import concourse.mybir as mybir
import concourse.tile as tile
from concourse import bacc
from concourse.bass_utils import run_bass_kernel_spmd

N_CORES = 8

# Compute dtype for the matmuls:
#   "bf16" - weights/activations cast to bfloat16 (f32 PSUM accumulate)
#   "f32r" - fp32 data, PE's replicated-fp32 mode (full rate at N>=256)
#   "f32"  - plain fp32 matmuls (4x slower PE)
MODE = os.environ.get("BASS_MOE_MODE", "bf16")

FG = int(os.environ.get("BASS_MOE_FG", "4"))  # steady-state f-chunks per W1 group
W2P = int(os.environ.get("BASS_MOE_W2P", "4"))  # f-chunks per W2 pair-tile
WARM = int(os.environ.get("BASS_MOE_WARM", "9"))  # dummy warmup matmuls (>=3.4us
# of continuous PE activity so the HAM clock-gate opens before the first stall)
CMAX = int(os.environ.get("BASS_MOE_CMAX", "256"))  # device capacity cap
LEADS = [int(v) for v in os.environ.get("BASS_MOE_LEADS", "1,1,2").split(",")]


def _w1_groups(KF):
    """F-chunk widths per W1 group: small leading groups let the PE start
    before the whole first FG-wide image lands, and keep per-chunk data
    availability fine-grained while the 8 cores' synchronized opening DMA
    burst halves the per-core HBM share (a group is usable only when its
    whole transfer lands)."""
    lead = list(LEADS) if FG > 2 and KF > 12 else []
    rem = KF - sum(lead)
    groups = list(lead)
    while rem > 0:
        w = min(FG, rem)
        groups.append(w)
        rem -= w
    return groups


_CACHE: dict = {}
_WCACHE: dict = {}


def _roundup(a: int, m: int) -> int:
    return -(-a // m) * m


def _build_bass(C: int, mode: str, D: int, F: int):
    """Build + compile the per-core Bass program for capacity C (<=512)."""
    f32 = mybir.dt.float32
    if mode == "bf16":
        dt_io = mybir.dt.bfloat16
    elif mode == "f32r":
        dt_io = mybir.dt.float32r
    else:
        dt_io = f32

    KD, KF = D // 128, F // 128
    GRPS = _w1_groups(KF)
    NP = -(-KF // W2P)  # number of W2 pair-tiles
    assert 256 <= C <= 512

    nc = bacc.Bacc(None, target_bir_lowering=False, debug=False)
    # Host-packed images (see kernel() for the packing). All contiguous
    # partition-major so per-partition descriptors are large.
    #   xP   [128, KD*C]    xP[p, d*C+c] = x[c, d*128+p]
    #   w1   [128, KD*F]    group images; group g at column offset
    #                       KD*128*sum(GRPS[:g]), blocks (d, j) within a
    #                       group at (d*gw+j)*128
    #   w2   [NP, 128, W2P*D]  pair p, f-chunk r=f-p*W2P at cols r*D
    #   b1r  [128, KF]      b1[f*128+p] at [p, f]
    #   yP   [128, KD*C]    output, same layout as xP (io dtype)
    xP = nc.dram_tensor("xP", [128, KD * C], dt_io, kind="ExternalInput")
    w1 = nc.dram_tensor("w1", [128, KD * F], dt_io, kind="ExternalInput")
    w2 = nc.dram_tensor("w2", [NP, 128, W2P * D], dt_io, kind="ExternalInput")
    b1r = nc.dram_tensor("b1r", [128, KF], f32, kind="ExternalInput")
    yP = nc.dram_tensor("yP", [128, KD * C], dt_io, kind="ExternalOutput")

    silu = mybir.ActivationFunctionType.Silu

    with tile.TileContext(nc) as tc:
        with (
            tc.tile_pool(name="xp", bufs=1) as xp,
            tc.tile_pool(name="w1p", bufs=4) as w1p,
            tc.tile_pool(name="w2p", bufs=1) as w2p,
            tc.tile_pool(name="hp", bufs=1) as hp,
            tc.tile_pool(name="bp", bufs=1) as bp,
            tc.tile_pool(name="yp", bufs=6) as yp,
            tc.tile_pool(name="wup", bufs=1) as wup,
            tc.tile_pool(name="ps_h", bufs=2, space="PSUM") as ps_h,
            tc.tile_pool(name="ps_y", bufs=1, space="PSUM") as ps_y,
        ):
            # PE warmup: dummy matmuls on a zeroed scratch tile keep the PE
            # busy (and ramping to 2.4 GHz) while the first x/W1 DMAs land.
            # The dummies borrow a ps_h buffer; the first real mm1 chunk's
            # start=True overwrites it (WAW-ordered on the in-order PE).
            if WARM > 0:
                wt = wup.tile([128, 512], dt_io, tag="warm", name="warm")
                nc.vector.memset(wt[:], 0.0)
                pw = ps_h.tile([128, 512], f32, tag="hps", name="pw")
                for _ in range(WARM):
                    nc.tensor.matmul(pw[:], wt[:, 0:128], wt[:], start=True, stop=True)

            # All bulk weight traffic rides the sync ring in consumption
            # order (W1 g0..gN, then W2 p0..pN): one ring drains FIFO at
            # full aggregate bandwidth, so nothing round-robins against the
            # critical W1 stream. x goes on the scalar ring so its trigger
            # issues in parallel with g0's (the first matmul gates on BOTH);
            # scalar afterwards carries only b1 (tiny) + the late y-outs.
            xt = xp.tile([128, KD * C], dt_io, tag="x", name="xt")
            nc.scalar.dma_start(out=xt[:], in_=xP[:])
            b1t = bp.tile([128, KF], f32, tag="b1", name="b1t")
            nc.scalar.dma_start(out=b1t[:], in_=b1r[:])

            w2ts: list = [None] * NP

            def load_w2(p):
                t = w2p.tile([128, W2P * D], dt_io, tag=f"w2_{p}", name=f"w2t{p}")
                nc.sync.dma_start(out=t[:], in_=w2[p])
                w2ts[p] = t

            # phase 1: hT[f] = silu(W1[:, f]^T @ xT + b1) for all f
            hts: list = [None] * KF
            f0 = 0
            for g, gw in enumerate(GRPS):
                off = KD * 128 * f0
                w1g = w1p.tile(
                    [128, KD * gw * 128],
                    dt_io,
                    tag="w1g",
                    name=f"w1g{g}",
                    padded_shape=[128, KD * FG * 128],
                )
                nc.sync.dma_start(out=w1g[:], in_=w1[:, off : off + KD * gw * 128])
                for j in range(gw):
                    f = f0 + j
                    ph = ps_h.tile([128, C], f32, tag="hps", name="ph")
                    for d in range(KD):
                        nc.tensor.matmul(
                            ph[:],
                            w1g[:, (d * gw + j) * 128 : (d * gw + j + 1) * 128],
                            xt[:, d * C : (d + 1) * C],
                            start=(d == 0),
                            stop=(d == KD - 1),
                        )
                    ht = hp.tile([128, C], dt_io, tag=f"h{f}", name=f"ht{f}")
                    nc.scalar.activation(ht[:], ph[:], silu, bias=b1t[:, f : f + 1])
                    hts[f] = ht
                f0 += gw
            # W2 pairs emitted after the W1 stream: the scheduler issues the
            # sync ring's triggers in priority (= program) order, so these
            # queue behind every W1 group and drain in consumption order.
            for p in range(NP):
                load_w2(p)

            # phase 2: yT[d] = sum_f W2[f, d]^T @ hT[f].
            # Pair-major order so W2 pair p is first touched only after
            # p * KD * W2P * C PE-cycles of phase 2 — consumption deadlines
            # match the sync ring's FIFO arrival order with maximum slack
            # (dd-major would need ALL of W2 resident at phase-2 start).
            # Each d-chunk drains (PSUM->SBUF->HBM) as its accumulation
            # completes in the last pair's pass.
            pys = [
                ps_y.tile([128, C], f32, tag=f"y{dd}", name=f"py{dd}")
                for dd in range(KD)
            ]

            def mm2(p, dd, r):
                f = p * W2P + r
                nc.tensor.matmul(
                    pys[dd][:],
                    w2ts[p][:, r * D + dd * 128 : r * D + (dd + 1) * 128],
                    hts[f][:],
                    start=(f == 0),
                    stop=(f == KF - 1),
                )

            tail_pairs = 2 if NP >= 3 else 1
            for p in range(NP - tail_pairs):
                for dd in range(KD):
                    for r in range(W2P):
                        mm2(p, dd, r)
            # last pairs run dd-major so each d-chunk's accumulation finishes
            # (stop=True) a block early and its drain staggers under the
            # remaining matmuls
            for dd in range(KD):
                for p in range(NP - tail_pairs, NP):
                    for r in range(W2P):
                        mm2(p, dd, r)
                yt = yp.tile([128, C], dt_io, tag="yt", name=f"yt{dd}")
                # last d-chunk's evac on ACT so it overlaps DVE finishing
                # the previous one; out-DMA triggers alternate rings
                if dd == KD - 1:
                    nc.scalar.copy(yt[:], pys[dd][:])
                else:
                    nc.vector.tensor_copy(yt[:], pys[dd][:])
                out_eng = nc.scalar if dd % 2 == 0 else nc.sync
                out_eng.dma_start(out=yP[:, dd * C : (dd + 1) * C], in_=yt[:])

    nc.compile()
    return nc


def _get_bass(C: int, mode: str, D: int, F: int):
    key = (C, mode, D, F, FG, W2P, WARM, tuple(LEADS))
    if key not in _CACHE:
        _CACHE[key] = _build_bass(C, mode, D, F)
    return _CACHE[key]


def _gate_host(x: np.ndarray, Wg: np.ndarray):
    """Top-1 gating in float64: returns (expert_idx [T], gate [T] f32)."""
    logits = x.astype(np.float64) @ Wg.astype(np.float64)
    m = logits.max(-1, keepdims=True)
    p = np.exp(logits - m)
    p /= p.sum(-1, keepdims=True)
    return p.argmax(-1), p.max(-1).astype(np.float32)


def _kernel_numpy(x, Wg, W1, b1, W2, b2):
    """Reference-equivalent fallback (host only)."""
    idx, gate = _gate_host(x, Wg)
    out = np.zeros_like(x)
    for e in range(W1.shape[0]):
        ids = np.nonzero(idx == e)[0]
        if ids.size == 0:
            continue
        h = x[ids] @ W1[e] + b1[e]
        h = h * (1.0 / (1.0 + np.exp(-h)))
        out[ids] = gate[ids, None] * (h @ W2[e] + b2[e])
    return out


def _pack_weights(W1, b1, W2, np_io, D, F):
    """Per-expert weight images (cached across calls on array identity)."""
    key = (id(W1), id(W2), id(b1), np_io)
    ent = _WCACHE.get(key)
    if ent is not None and ent[0] is W1 and ent[1] is W2 and ent[2] is b1:
        return ent[3]
    E = W1.shape[0]
    KD, KF = D // 128, F // 128
    NP = -(-KF // W2P)
    grps = _w1_groups(KF)
    packed = []
    for e in range(E):
        w1e = W1[e].reshape(KD, 128, KF, 128)
        parts = []
        f0 = 0
        for gw in grps:
            blk = w1e[:, :, f0 : f0 + gw]  # [KD, 128, gw, 128]
            parts.append(blk.transpose(1, 0, 2, 3).reshape(128, KD * gw * 128))
            f0 += gw
        w1r = np.concatenate(parts, axis=1)  # [128, KD*F]
        w2r = (
            W2[e]
            .reshape(NP, W2P, 128, D)
            .transpose(0, 2, 1, 3)
            .reshape(NP, 128, W2P * D)
        )
        packed.append(
            {
                "w1": np.ascontiguousarray(w1r).astype(np_io, copy=False),
                "w2": np.ascontiguousarray(w2r).astype(np_io, copy=False),
                "b1r": np.ascontiguousarray(b1[e].reshape(KF, 128).T),
            }
        )
    _WCACHE[key] = (W1, W2, b1, packed)
    return packed


def kernel(hidden_states, Wg, W1, b1, W2, b2):
    hidden_states = np.asarray(hidden_states)
    Wg = np.asarray(Wg, dtype=np.float32)
    W1 = np.asarray(W1, dtype=np.float32)
    b1 = np.asarray(b1, dtype=np.float32)
    W2 = np.asarray(W2, dtype=np.float32)
    b2 = np.asarray(b2, dtype=np.float32)

    orig_shape = hidden_states.shape
    D = orig_shape[-1]
    x = np.ascontiguousarray(hidden_states, dtype=np.float32).reshape(-1, D)
    E, _, F = W1.shape
    KD, KF = D // 128, F // 128

    if E != N_CORES or D % 128 != 0 or F % 128 != 0:
        return _kernel_numpy(x, Wg, W1, b1, W2, b2).reshape(orig_shape)

    idx, gate = _gate_host(x, Wg)
    order = np.argsort(idx, kind="stable")
    counts = np.bincount(idx, minlength=E)
    starts = np.concatenate([[0], np.cumsum(counts)])

    # Capacity: common padded token count per core (single slab, <=512 to fit
    # one PSUM bank per matmul output). Capacity is capped at CMAX (the
    # balanced load T/E) and the few overflow tokens of hot experts are
    # FFN'd on the host — classic capacity-factor MoE with a residual
    # correction instead of token dropping.
    C = max(256, _roundup(int(counts.max()), 16))
    C = min(C, max(256, CMAX))
    if C > 512:
        return _kernel_numpy(x, Wg, W1, b1, W2, b2).reshape(orig_shape)

    mode = MODE
    np_io = np.float32
    if mode == "bf16":
        import ml_dtypes

        np_io = ml_dtypes.bfloat16

    nc = _get_bass(C, mode, D, F)
    wpack = _pack_weights(W1, b1, W2, np_io, D, F)

    in_maps = []
    for e in range(E):
        ids = order[starts[e] : starts[e + 1]][:C]
        xe = np.zeros((C, D), dtype=np.float32)
        xe[: ids.size] = x[ids]
        # xP[p, d*C+c] = xe[c, d*128+p]
        xPr = xe.reshape(C, KD, 128).transpose(2, 1, 0).reshape(128, KD * C)
        m = dict(wpack[e])
        m["xP"] = np.ascontiguousarray(xPr).astype(np_io, copy=False)
        in_maps.append(m)

    res = run_bass_kernel_spmd(nc, in_maps, list(range(N_CORES)))

    out = np.zeros_like(x)
    for e in range(E):
        ids = order[starts[e] : starts[e + 1]][:C]
        if ids.size:
            yr = np.asarray(res.results[e]["yP"], dtype=np.float32)  # [128, KD*C]
            y = yr.reshape(128, KD, C).transpose(2, 1, 0).reshape(C, D)[: ids.size]
            out[ids] = gate[ids, None] * (y + b2[e])
        over = order[starts[e] + C : starts[e + 1]]
        if over.size:  # host FFN for tokens past capacity
            h = x[over] @ W1[e] + b1[e]
            h = h * (1.0 / (1.0 + np.exp(-h)))
            out[over] = gate[over, None] * (h @ W2[e] + b2[e])
    return out.reshape(orig_shape)
